# revision 1
# baseline (speedup 1.0000x reference)
"""Trainium2 Bass kernel for segmented linear (performer-style) attention.

Problem: nn_Attention_43550968382196 (sparse_attention).
  N=32768 tokens in 64 contiguous equal segments of 512, d_qk=128, d_v=256,
  m=256 random features.  Per segment:
     phi_q = (exp(Uq - hq - rowmax(Uq)) + eps) / sqrt(m)
     phi_k = (exp(Uk - hk - segmax(Uk)) + eps) / sqrt(m)
     out   = (phi_q @ (phi_k^T V)) / (phi_q . sum(phi_k) + 1e-8)

Device math (all equivalent to the reference up to ~1e-6):
  * 1/sqrt(m) cancels in the ratio -> unscaled phi, eps_norm' = 1e-8*m.
  * exp runs BEFORE the max; rowmax(exp(U)) == exp(rowmax U) by
    monotonicity, so the stabilizer is a multiplicative per-row scale.
  * The K side is left unnormalized by the segment max (it cancels in the
    ratio): Kp~ = exp(Uk)*exp(-hk).  The phi-eps term then needs
    eps*segmax, which is folded in as a rank-1 PE-accumulated correction
    KV += (eps*segmax) * ones ⊗ colsum([V|1]), and the normalizer eps
    becomes (1e-8*m)*segmax, broadcast via a tiny ones-matmul.  This keeps
    the cross-partition segment-max reduction off the critical path.
  * fp32r (11-bit-mantissa fp32) operands for all big matmuls; V/omega/Q^T/
    K^T are pre-rounded on the host, Qp/Kp/KV round on the producing engine.

Sharding: 64 segments split 8-per-core across 8 NeuronCores (data parallel,
no collectives); each core runs this program on its 4096-token shard.
"""

import math
import os
import sys

for _p in ("/opt/trn_rl_repo",):
    if _p not in sys.path and os.path.isdir(_p):
        sys.path.insert(0, _p)

import numpy as np

import concourse.bass as bass
import concourse.bacc as bacc
import concourse.tile as tile
from concourse import mybir
from concourse.bass_utils import run_bass_kernel_spmd

F32 = mybir.dt.float32
F32R = mybir.dt.float32r
AF = mybir.ActivationFunctionType
ALU = mybir.AluOpType
AX = mybir.AxisListType

N_CORES = 8
N = 32768
D = 128          # qk dim
M = 256          # features
DV = 256         # v dim
DVA = 258        # device V columns: [V | 1 | 0] (fp32r needs even N)
P = 128          # partitions / tokens per chunk
NSEG = int(os.environ.get('KERNEL_NSEG', 8))  # segments per core
CH = 4           # chunks per segment
MC = 2           # m chunks (256 / 128)
TOK = NSEG * 512

EPS_PHI = 1e-4
EPS_NORM2 = 1e-8 * M
H_SCALE = 1.0 / (2.0 * math.sqrt(D))
SQ2 = H_SCALE ** 0.5          # Square(x*SQ2) = x^2/(2 sqrt d)


def build_nc():
    nc = bacc.Bacc("TRN2", target_bir_lowering=False, debug=False)

    HQd = nc.declare_dram_parameter("HQK", [P, NSEG * CH * 2], F32,
                                    isOutput=False)
    QTd = nc.declare_dram_parameter("QT", [D, TOK], F32R, isOutput=False)
    KTd = nc.declare_dram_parameter("KT", [D, TOK], F32R, isOutput=False)
    Vd = nc.declare_dram_parameter("V", [TOK, DVA], F32R, isOutput=False)
    Wd = nc.declare_dram_parameter("omega", [D, M], F32R, isOutput=False)
    Id = nc.declare_dram_parameter("ident", [P, P], F32, isOutput=False)
    Ir = nc.declare_dram_parameter("identr", [P, P], F32R, isOutput=False)
    Ord = nc.declare_dram_parameter("onesr", [1, P], F32R, isOutput=False)
    Ocd = nc.declare_dram_parameter("onesc", [P, 1], F32R, isOutput=False)
    Od = nc.declare_dram_parameter("out", [TOK, DV], F32, isOutput=True)

    Vv = Vd[:, :].rearrange("(s c p) d -> s p c d", s=NSEG, c=CH, p=P)
    Ov = Od[:, :].rearrange("(s c p) d -> s p c d", s=NSEG, c=CH, p=P)

    with tile.TileContext(nc) as tc:
        with (
            tc.tile_pool(name="const", bufs=1) as const,
            tc.tile_pool(name="sb", bufs=2) as sb,
            tc.tile_pool(name="sm", bufs=3) as sm,
            tc.tile_pool(name="ps", bufs=1, space="PSUM") as ps,
        ):
            omega_t = const.tile([D, M], F32R, name="omega_t")
            nc.sync.dma_start(omega_t[:, :], Wd[:, :])
            ident_t = const.tile([P, P], F32, name="ident_t")
            nc.sync.dma_start(ident_t[:, :], Id[:, :])
            ident_r = const.tile([P, P], F32R, name="ident_r")
            nc.sync.dma_start(ident_r[:, :], Ir[:, :])
            ones_row = const.tile([1, P], F32, name="ones_row")
            nc.vector.memset(ones_row[:, :], 1.0)
            onesr_t = const.tile([1, P], F32R, name="onesr_t")
            nc.sync.dma_start(onesr_t[:, :], Ord[:, :])
            onesc_t = const.tile([P, 1], F32R, name="onesc_t")
            nc.sync.dma_start(onesc_t[:, :], Ocd[:, :])
            # per-segment slices so segment 0 compute starts right away
            qT_all = const.tile([D, TOK], F32R, name="qT_all")
            kT_all = const.tile([D, TOK], F32R, name="kT_all")
            for s in range(NSEG):
                sl = bass.ts(s, 512)
                nc.sync.dma_start(qT_all[:, sl], QTd[:, sl])
                nc.sync.dma_start(kT_all[:, sl], KTd[:, sl])
            hq_all = const.tile([P, NSEG, CH, 2], F32, name="hq_all")
            nc.sync.dma_start(
                hq_all[:, :, :, :],
                HQd[:, :].rearrange("p (s c t) -> p s c t", s=NSEG, c=CH))


            for s in range(NSEG):
                vt = sb.tile([P, CH, DVA], F32R, name=f"vt{s}", tag="vt",
                             bufs=4)
                nc.sync.dma_start(vt[:, :, :], Vv[s])
                hqk = hq_all[:, s]

                # ---- U matmuls (lhsT slices of preloaded Q^T/K^T) -------
                uq0 = ps.tile([P, 2, M], F32, name=f"uq0_{s}", tag="U", bufs=3)
                uq1 = ps.tile([P, 2, M], F32, name=f"uq1_{s}", tag="U", bufs=3)
                uk0 = ps.tile([P, 2, M], F32, name=f"uk0_{s}", tag="U", bufs=3)
                uk1 = ps.tile([P, 2, M], F32, name=f"uk1_{s}", tag="U", bufs=3)
                uqh = (uq0, uq1)
                ukh = (uk0, uk1)
                for c in range(CH):
                    nc.tensor.matmul(uqh[c // 2][:, c % 2, :],
                                     qT_all[:, bass.ts(s * CH + c, P)],
                                     omega_t[:, :])
                    nc.tensor.matmul(ukh[c // 2][:, c % 2, :],
                                     kT_all[:, bass.ts(s * CH + c, P)],
                                     omega_t[:, :])

                # ---- exp: eq0 = exp(Uq) raw; ek1 = exp(Uk - hk) ---------
                eq0 = sb.tile([P, CH, M], F32, name=f"eq0_{s}", tag="eq0", bufs=4)
                for hf in range(2):
                    nc.scalar.activation(eq0[:, 2 * hf:2 * hf + 2, :],
                                         uqh[hf][:, :, :], AF.Exp)
                ek1 = sb.tile([P, CH, M], F32R, name=f"ek1_{s}", tag="ek1", bufs=4)
                for c in range(CH):
                    nc.scalar.activation(ek1[:, c, :], ukh[c // 2][:, c % 2, :],
                                         AF.Exp, bias=hqk[:, c, 1:2])

                # ---- maxes from raw U (PSUM) ----------------------------
                xmq = sm.tile([P, CH], F32, name=f"xmq{s}", tag="xmq")
                nc.vector.tensor_reduce(xmq[:, 0:2], uq0[:, :, :],
                                        axis=AX.X, op=ALU.max)
                nc.vector.tensor_reduce(xmq[:, 2:4], uq1[:, :, :],
                                        axis=AX.X, op=ALU.max)
                xmk2 = sm.tile([P, 2], F32, name=f"xmk2_{s}", tag="xmk2")
                nc.vector.tensor_reduce(xmk2[:, 0:1], uk0[:, :, :],
                                        axis=AX.XY, op=ALU.max)
                nc.vector.tensor_reduce(xmk2[:, 1:2], uk1[:, :, :],
                                        axis=AX.XY, op=ALU.max)
                xmk = sm.tile([P, 1], F32, name=f"xmk{s}", tag="xmk")
                nc.vector.tensor_tensor(xmk[:, :], xmk2[:, 0:1],
                                        xmk2[:, 1:2], op=ALU.max)
                # segment max -> scalar (PE transpose + reduce); feeds only
                # the eps corrections, off the critical path
                mkT = ps.tile([1, 512], F32, name=f"mkT{s}", tag="S", bufs=1)
                nc.tensor.transpose(mkT[0:1, 0:P], xmk[:, 0:1], ident_t[:, :])
                mkrow = sm.tile([1, P], F32, name=f"mkrow{s}", tag="mkrow")
                nc.vector.tensor_copy(mkrow[:, :], mkT[0:1, 0:P])
                msr = sm.tile([1, 1], F32, name=f"msr{s}", tag="msr")
                nc.vector.tensor_reduce(msr[:, :], mkrow[:, :], axis=AX.X,
                                        op=ALU.max)
                mks = sm.tile([1, 1], F32, name=f"mks{s}", tag="mks")
                nc.scalar.activation(mks[:, :], msr[:, :], AF.Exp)

                # Vsum = colsum([V|1|0]) via ones-column matmul (PE)
                vsum = ps.tile([1, 512], F32, name=f"vsum{s}", tag="S",
                               bufs=1)
                for c in range(CH):
                    nc.tensor.matmul(vsum[0:1, 0:DVA], vt[:, c, DV:DV + 1],
                                     vt[:, c, :], start=(c == 0),
                                     stop=(c == CH - 1))
                # cvs = (eps_phi * segmax) * Vsum   [1, DVA] fp32r
                ceps = sm.tile([1, 1], F32, name=f"ceps{s}", tag="ceps")
                nc.vector.tensor_scalar_mul(ceps[:, :], mks[:, :], EPS_PHI)
                cvs = sm.tile([1, DVA], F32R, name=f"cvs{s}", tag="cvs")
                nc.vector.tensor_scalar_mul(cvs[:, :], vsum[0:1, 0:DVA],
                                            ceps[0:1, 0:1])
                # eps_norm * segmax broadcast to all partitions (PE)
                cen = sm.tile([1, 1], F32, name=f"cen{s}", tag="cen")
                nc.vector.tensor_scalar_mul(cen[:, :], mks[:, :], EPS_NORM2)
                enb = ps.tile([P, 512], F32, name=f"enb{s}", tag="S", bufs=1)
                nc.tensor.matmul(enb[:, 0:1], ones_row[:, :], cen[:, :])
                enb_sb = sm.tile([P, 1], F32, name=f"enbsb{s}", tag="enbsb")
                nc.vector.tensor_copy(enb_sb[:, :], enb[:, 0:1])

                # ---- Qp = eq0 * exp(-hq - mq) + eps ---------------------
                sqa = sm.tile([P, CH], F32, name=f"sqa{s}", tag="sqa")
                nc.vector.tensor_tensor(sqa[:, :], hqk[:, :, 0], xmq[:, :],
                                        op=ALU.subtract)
                sqv = sm.tile([P, CH], F32, name=f"sqv{s}", tag="sqv")
                nc.scalar.activation(sqv[:, :], sqa[:, :], AF.Exp)
                qp = sb.tile([P, CH, M], F32R, name=f"qp{s}", tag="qp", bufs=4)
                for c in range(CH):
                    nc.vector.tensor_scalar(qp[:, c, :], eq0[:, c, :],
                                            sqv[:, c:c + 1], EPS_PHI,
                                            op0=ALU.mult, op1=ALU.add)

                # ---- KV = Kp~^T @ [V|1|0]  (+ rank-1 eps correction) ----
                kv_sb = sb.tile([P, MC, DVA], F32R, name=f"kvsb{s}",
                                tag="kvsb", bufs=4)
                for mc in range(MC):
                    kvp = ps.tile([P, 512], F32, name=f"kv{s}_{mc}", tag="W",
                                  bufs=4)
                    for c in range(CH):
                        nc.tensor.matmul(kvp[:, 0:DVA],
                                         ek1[:, c, bass.ts(mc, P)],
                                         vt[:, c, :],
                                         start=(c == 0), stop=False)
                    nc.tensor.matmul(kvp[:, 0:DVA], onesr_t[0:1, :],
                                     cvs[0:1, :], start=False, stop=True)
                    if (mc + s) % 2 == 0:
                        nc.scalar.copy(kv_sb[:, mc, :], kvp[:, 0:DVA])
                    else:
                        nc.vector.tensor_copy(kv_sb[:, mc, :], kvp[:, 0:DVA])

                # ---- Qp^T (PE transpose) --------------------------------
                qpT_sb = sb.tile([P, MC, 512], F32R, name=f"qpTsb{s}",
                                 tag="qpTsb", bufs=4)
                for mc in range(MC):
                    qpTp = ps.tile([P, 512], F32R, name=f"qpT{s}_{mc}",
                                   tag="W", bufs=4)
                    for c in range(CH):
                        nc.tensor.transpose(qpTp[:, bass.ts(c, P)],
                                            qp[:, c, bass.ts(mc, P)],
                                            ident_r[:, :])
                    nc.scalar.copy(qpT_sb[:, mc, :], qpTp[:, :])

                # ---- num = Qp @ [KV | Ksum], per chunk ------------------
                ot = sb.tile([P, CH, DV], F32, name=f"ot{s}", tag="ot",
                             bufs=4)
                for c in range(CH):
                    nm = ps.tile([P, 512], F32, name=f"nm{s}_{c}",
                                 tag="W", bufs=4)
                    for mc in range(MC):
                        nc.tensor.matmul(nm[:, 0:DVA],
                                         qpT_sb[:, mc, bass.ts(c, P)],
                                         kv_sb[:, mc, :],
                                         start=(mc == 0),
                                         stop=(mc == MC - 1))
                    den = sm.tile([P, 1], F32, name=f"den{s}_{c}", tag="den")
                    nc.vector.tensor_scalar_add(den[:, :],
                                                nm[:, DV:DV + 1],
                                                enb_sb[:, 0:1])
                    rr = sm.tile([P, 1], F32, name=f"rr{s}_{c}", tag="rr")
                    nc.vector.reciprocal(rr[:, :], den[:, :])
                    if (c + s) % 2 == 0:
                        rrb = rr[:, :].broadcast_to([P, DV])
                        nc.vector.tensor_tensor(
                            ot[:, c, :], nm[:, 0:DV], rrb, op=ALU.mult)
                    else:
                        nc.scalar.activation(ot[:, c, :], nm[:, 0:DV],
                                             AF.Copy, scale=rr[:, 0:1])

                nc.sync.dma_start(Ov[s], ot[:, :, :])

    nc.compile()
    return nc


_NC_CACHE = {}


def _get_nc():
    if "nc" not in _NC_CACHE:
        _NC_CACHE["nc"] = build_nc()
    return _NC_CACHE["nc"]


def _round_f32r(x):
    xi = np.ascontiguousarray(x, np.float32).view(np.uint32)
    return ((xi + np.uint32(1 << 11)) & np.uint32(0xFFFFF000)).view(np.float32)


def make_in_maps(Q, K, V, omega):
    Q = np.ascontiguousarray(np.asarray(Q, dtype=np.float32))
    K = np.ascontiguousarray(np.asarray(K, dtype=np.float32))
    QT = _round_f32r(Q.T)
    KT = _round_f32r(K.T)
    hscale = np.float32(1.0 / (2.0 * math.sqrt(D)))
    hq = -(Q * Q).sum(axis=1) * hscale
    hk = -(K * K).sum(axis=1) * hscale
    # device layout [P, (s c t)] with token = (s*CH + c)*P + p per core
    hqk2 = np.stack([hq, hk], axis=1)          # [N, 2]
    V = np.asarray(V, dtype=np.float32)
    Vaug = np.zeros((V.shape[0], DVA), np.float32)
    Vaug[:, :DV] = _round_f32r(V)
    Vaug[:, DV] = 1.0
    omega = np.asarray(omega, dtype=np.float32)
    omega_s = _round_f32r(omega * np.float32(D ** -0.25))
    ident = np.eye(P, dtype=np.float32)
    ones_r = np.ones((1, P), np.float32)
    ones_c = np.ones((P, 1), np.float32)
    in_maps = []
    for c in range(N_CORES):
        sl = slice(c * TOK, (c + 1) * TOK)
        in_maps.append({
            "V": Vaug[sl],
            "HQK": np.ascontiguousarray(
                hqk2[sl].reshape(NSEG, CH, P, 2)
                .transpose(2, 0, 1, 3).reshape(P, NSEG * CH * 2)),
            "QT": np.ascontiguousarray(QT[:, sl]),
            "KT": np.ascontiguousarray(KT[:, sl]),
            "omega": omega_s, "ident": ident, "identr": ident,
            "onesr": ones_r, "onesc": ones_c,
        })
    return in_maps


def kernel(Q, K, V, omega, num_batch, batch_seg):
    nc = _get_nc()
    in_maps = make_in_maps(Q, K, V, omega)
    res = run_bass_kernel_spmd(nc, in_maps, core_ids=list(range(N_CORES)))
    return np.concatenate([res.results[c]["out"] for c in range(N_CORES)],
                          axis=0)



# revision 8
# speedup vs baseline: 1.2768x; 1.2768x over previous
"""Trainium2 Bass kernel for segmented linear (performer-style) attention.

Problem: nn_Attention_43550968382196 (sparse_attention).
  N=32768 tokens in 64 contiguous equal segments of 512, d_qk=128, d_v=256,
  m=256 random features.  Per segment:
     phi_q = (exp(Uq - hq - rowmax(Uq)) + eps) / sqrt(m)
     phi_k = (exp(Uk - hk - segmax(Uk)) + eps) / sqrt(m)
     out   = (phi_q @ (phi_k^T V)) / (phi_q . sum(phi_k) + 1e-8)

Device math (v5; validated 4.8e-3 rel err vs the jax reference in numpy):
  * All matmuls bf16 operands, fp32 PSUM accumulation (fp8 was tested and
    fails the 2e-2 gate: e4m3's 6% per-element error survives averaging).
  * Q side: Qp = exp(Uq - hq - mx) via one Act pass per chunk (bias AP);
    the +eps rides the PSUM->SBUF copy after the PE transpose (Copy with
    float bias / tensor_scalar_add), so no eps rank-1 on the Q side.
  * K side: e^{-hk} is folded into V rows ON THE HOST (V' = e^{-hk} V), so
    phi_k-dev = exp(Uk) needs NO bias: one exp covers 2 chunks.  The
    segment max enters only through the eps correction: segmax' =
    max(exp(Uk)) = e^{segmax} via a gpsimd all-dims reduce, used as the
    rank-1 lhsT scale: KV += segmax' * (eps * colsum_raw(V)), Ksum +=
    segmax' * eps*512.  Per-segment scale e^{segmax} cancels in the ratio.
  * den is a separate 1-column matmul chain; num/den are DMA'd out in raw
    fp32 straight from PSUM and the division (+ the 1e-8*m*segmax' norm
    epsilon) happens on the host.

Sharding: 64 segments split 8-per-core across 8 NeuronCores (data parallel,
no collectives); each core runs this program on its 4096-token shard.
"""

import math
import os
import sys

for _p in ("/opt/trn_rl_repo",):
    if _p not in sys.path and os.path.isdir(_p):
        sys.path.insert(0, _p)

import numpy as np
import ml_dtypes

import concourse.bass as bass
import concourse.bacc as bacc
import concourse.tile as tile
from concourse import mybir
from concourse.bass_utils import run_bass_kernel_spmd

F32 = mybir.dt.float32
BF16 = mybir.dt.bfloat16
AF = mybir.ActivationFunctionType
ALU = mybir.AluOpType
AX = mybir.AxisListType

N_CORES = 8
N = 32768
D = 128          # qk dim
M = 256          # features
DV = 256         # v dim
P = 128          # partitions / tokens per chunk
NSEG = 8         # segments per core
CH = 4           # chunks per segment
MC = 2           # m chunks (256 / 128)
TOK = NSEG * 512

EPS_PHI = 1e-4
EPS_NORM = 1e-8


def build_nc():
    nc = bacc.Bacc("TRN2", target_bir_lowering=False, debug=False)

    QTd = nc.declare_dram_parameter("QT", [D, TOK], BF16, isOutput=False)
    KTd = nc.declare_dram_parameter("KT", [D, TOK], BF16, isOutput=False)
    Vd = nc.declare_dram_parameter("VP", [TOK, DV], BF16, isOutput=False)
    Wd = nc.declare_dram_parameter("omega", [D, M], BF16, isOutput=False)
    HQd = nc.declare_dram_parameter("HQM", [P, NSEG * CH], F32, isOutput=False)
    EHd = nc.declare_dram_parameter("EHK", [P, NSEG * CH], BF16, isOutput=False)
    CVd = nc.declare_dram_parameter("CVS", [1, NSEG * DV], BF16, isOutput=False)
    Id = nc.declare_dram_parameter("identb", [P, P], BF16, isOutput=False)
    Od = nc.declare_dram_parameter("num", [TOK, DV], BF16, isOutput=True)
    Dd = nc.declare_dram_parameter("den", [P, NSEG * CH], F32, isOutput=True)
    Sd = nc.declare_dram_parameter("smax", [1, NSEG], F32, isOutput=True)

    Vv = Vd[:, :].rearrange("(s c p) d -> s p c d", s=NSEG, c=CH, p=P)
    Ov = Od[:, :].rearrange("(s c p) d -> s p c d", s=NSEG, c=CH, p=P)

    with tile.TileContext(nc) as tc:
        with (
            tc.tile_pool(name="const", bufs=1) as const,
            tc.tile_pool(name="sb", bufs=2) as sb,
            tc.tile_pool(name="sm", bufs=3) as sm,
            tc.tile_pool(name="ps", bufs=1, space="PSUM") as ps,
        ):
            omega_t = const.tile([D, M], BF16, name="omega_t")
            nc.sync.dma_start(omega_t[:, :], Wd[:, :])
            ident_t = const.tile([P, P], BF16, name="ident_t")
            nc.sync.dma_start(ident_t[:, :], Id[:, :])
            hqm_t = const.tile([P, NSEG, CH], F32, name="hqm_t")
            nc.sync.dma_start(
                hqm_t[:, :, :],
                HQd[:, :].rearrange("p (s c) -> p s c", s=NSEG))
            ehk_t = const.tile([P, NSEG, CH], BF16, name="ehk_t")
            nc.sync.dma_start(
                ehk_t[:, :, :],
                EHd[:, :].rearrange("p (s c) -> p s c", s=NSEG))
            cvs_t = const.tile([1, NSEG, DV], BF16, name="cvs_t")
            nc.sync.dma_start(
                cvs_t[:, :, :],
                CVd[:, :].rearrange("p (s d) -> p s d", s=NSEG))
            c512_t = const.tile([1, 1], BF16, name="c512_t")
            nc.vector.memset(c512_t[:, :], EPS_PHI * 512.0)
            # outputs accumulated in SBUF, one DMA each at the end
            denAll = const.tile([P, NSEG, CH], F32, name="denAll")
            smaxAll = const.tile([1, NSEG], F32, name="smaxAll")

            # bulk input loads, segment 0 slices first so compute starts
            qT_all = const.tile([D, TOK], BF16, name="qT_all")
            kT_all = const.tile([D, TOK], BF16, name="kT_all")
            vp_all = const.tile([P, NSEG, CH, DV], BF16, name="vp_all")
            nc.sync.dma_start(qT_all[:, 0:512], QTd[:, 0:512])
            nc.sync.dma_start(kT_all[:, 0:512], KTd[:, 0:512])
            nc.sync.dma_start(vp_all[:, 0], Vv[0])
            nc.sync.dma_start(qT_all[:, 512:TOK], QTd[:, 512:TOK])
            nc.sync.dma_start(kT_all[:, 512:TOK], KTd[:, 512:TOK])
            for s in range(1, NSEG):
                nc.sync.dma_start(vp_all[:, s], Vv[s])

            # per-segment state carried between pipeline stages
            st = [None] * NSEG

            def stage1(s):
                # ---- U matmuls: psU tiles [P, 2, M], 1 bank each ------
                uq0 = ps.tile([P, 2, M], F32, name=f"uq0_{s}", tag="U", bufs=3)
                uq1 = ps.tile([P, 2, M], F32, name=f"uq1_{s}", tag="U", bufs=3)
                uk0 = ps.tile([P, 2, M], F32, name=f"uk0_{s}", tag="U", bufs=3)
                uk1 = ps.tile([P, 2, M], F32, name=f"uk1_{s}", tag="U", bufs=3)
                for c in range(CH):
                    u = (uq0, uq1)[c // 2]
                    nc.tensor.matmul(u[:, c % 2, :],
                                     qT_all[:, bass.ts(s * CH + c, P)],
                                     omega_t[:, :])
                for c in range(CH):
                    u = (uk0, uk1)[c // 2]
                    nc.tensor.matmul(u[:, c % 2, :],
                                     kT_all[:, bass.ts(s * CH + c, P)],
                                     omega_t[:, :])

                # ---- Q: rowmax -> bias -> exp (Act, bf16 out) ---------
                mx4 = sm.tile([P, CH], F32, name=f"mx4_{s}", tag="mx4")
                nc.vector.tensor_reduce(mx4[:, 0:2], uq0[:, :, :],
                                        axis=AX.X, op=ALU.max)
                nc.vector.tensor_reduce(mx4[:, 2:4], uq1[:, :, :],
                                        axis=AX.X, op=ALU.max)
                biasq = sm.tile([P, CH], F32, name=f"biasq_{s}", tag="biasq")
                nc.vector.tensor_tensor(biasq[:, :], hqm_t[:, s], mx4[:, :],
                                        op=ALU.subtract)
                qp = sb.tile([P, CH, M], BF16, name=f"qp{s}", tag="qp", bufs=3)
                for c in range(CH):
                    nc.scalar.activation(qp[:, c, :],
                                         (uq0, uq1)[c // 2][:, c % 2, :],
                                         AF.Exp, bias=biasq[:, c:c + 1])

                # ---- K: exp with no bias (one op per psU tile) --------
                kp = sb.tile([P, CH, M], BF16, name=f"kp{s}", tag="kp", bufs=3)
                nc.scalar.activation(kp[:, 0:2, :], uk0[:, :, :], AF.Exp)
                nc.scalar.activation(kp[:, 2:4, :], uk1[:, :, :], AF.Exp)

                # ---- segmax' = max(exp(Uk)) via gpsimd all-reduce -----
                smx = sm.tile([1, 1], F32, name=f"smx{s}", tag="smx")
                nc.gpsimd.tensor_reduce(smx[:, :], kp[:, :, :],
                                        axis=AX.XYZWC, op=ALU.max)
                smrow = sm.tile([1, P], BF16, name=f"smrow{s}", tag="smrow")
                nc.vector.tensor_copy(smrow[:, :],
                                      smx[:, :].broadcast_to([1, P]))
                nc.vector.tensor_copy(smaxAll[0:1, s:s + 1], smx[:, :])
                st[s] = (qp, kp, smrow)

            def stage2a(s):
                qp, kp, smrow = st[s]
                # ---- QpT = T(qp) + eps  (PE transpose, copy adds eps) -
                psT0 = ps.tile([P, 512], BF16, name=f"psT0_{s}", tag="T",
                               bufs=2)
                psT1 = ps.tile([P, 512], BF16, name=f"psT1_{s}", tag="T",
                               bufs=2)
                for c in range(CH):
                    nc.tensor.transpose(psT0[:, bass.ts(c, P)],
                                        qp[:, c, 0:P], ident_t[:, :])
                    nc.tensor.transpose(psT1[:, bass.ts(c, P)],
                                        qp[:, c, P:M], ident_t[:, :])
                qpT = sb.tile([P, MC, 512], BF16, name=f"qpT{s}", tag="qpT",
                              bufs=2)
                nc.scalar.activation(qpT[:, 0, :], psT0[:, :], AF.Copy,
                                     bias=EPS_PHI)
                nc.vector.tensor_scalar_add(qpT[:, 1, :], psT1[:, :],
                                            EPS_PHI)

                # ---- KV = Kp^T V' (+ rank-1 eps) ; Ksum likewise ------
                psKV = ps.tile([P, MC, DV], F32, name=f"psKV{s}", tag="W",
                               bufs=1)
                psKs = ps.tile([P, MC, 1], F32, name=f"psKs{s}", tag="T",
                               bufs=2)
                for mc in range(MC):
                    for c in range(CH):
                        nc.tensor.matmul(psKV[:, mc, :],
                                         kp[:, c, bass.ts(mc, P)],
                                         vp_all[:, s, c, :],
                                         start=(c == 0), stop=False)
                    nc.tensor.matmul(psKV[:, mc, :], smrow[0:1, :],
                                     cvs_t[0:1, s, :], start=False, stop=True)
                    for c in range(CH):
                        nc.tensor.matmul(psKs[:, mc, :],
                                         kp[:, c, bass.ts(mc, P)],
                                         ehk_t[:, s, c:c + 1],
                                         start=(c == 0), stop=False)
                    nc.tensor.matmul(psKs[:, mc, :], smrow[0:1, :],
                                     c512_t[0:1, :], start=False, stop=True)
                kvb = sb.tile([P, MC, DV + 1], BF16, name=f"kvb{s}",
                              tag="kvb", bufs=2)
                nc.scalar.activation(kvb[:, 0, 0:DV], psKV[:, 0, :], AF.Copy)
                nc.vector.tensor_copy(kvb[:, 1, 0:DV], psKV[:, 1, :])
                nc.vector.tensor_copy(kvb[:, :, DV:DV + 1], psKs[:, :, :])
                st[s] = (qpT, kvb)

            def stage2b(s):
                qpT, kvb = st[s]
                # ---- num / den matmuls + stores -----------------------
                psD = ps.tile([P, CH], F32, name=f"psD{s}", tag="T", bufs=2)
                for half in range(2):
                    psN = ps.tile([P, 2, DV], F32, name=f"psN{s}_{half}",
                                  tag="NN", bufs=2)
                    for i in range(2):
                        c = half * 2 + i
                        for mc in range(MC):
                            nc.tensor.matmul(psN[:, i, :],
                                             qpT[:, mc, bass.ts(c, P)],
                                             kvb[:, mc, 0:DV],
                                             start=(mc == 0), stop=(mc == 1))
                        for mc in range(MC):
                            nc.tensor.matmul(psD[:, c:c + 1],
                                             qpT[:, mc, bass.ts(c, P)],
                                             kvb[:, mc, DV:DV + 1],
                                             start=(mc == 0), stop=(mc == 1))
                    numb = sb.tile([P, 2, DV], BF16, name=f"numb{s}_{half}",
                                   tag="numb", bufs=3)
                    if half == 0:
                        nc.scalar.activation(numb[:, :, :], psN[:, :, :],
                                             AF.Copy)
                    else:
                        nc.vector.tensor_copy(numb[:, :, :], psN[:, :, :])
                    nc.sync.dma_start(Ov[s, :, 2 * half:2 * half + 2, :],
                                      numb[:, :, :])
                nc.vector.tensor_copy(denAll[:, s, :], psD[:, :])

            for s in range(NSEG):
                if s > 0:
                    stage2a(s - 1)
                stage1(s)
                if s > 0:
                    stage2b(s - 1)
            stage2a(NSEG - 1)
            stage2b(NSEG - 1)

            nc.sync.dma_start(Dd[:, :],
                              denAll[:, :, :].rearrange("p s c -> p (s c)"))
            nc.sync.dma_start(Sd[:, :], smaxAll[:, :])

    nc.compile()
    return nc


_NC_CACHE = {}


def _get_nc():
    if "nc" not in _NC_CACHE:
        _NC_CACHE["nc"] = build_nc()
    return _NC_CACHE["nc"]


def make_in_maps(Q, K, V, omega):
    bf = ml_dtypes.bfloat16
    Q = np.ascontiguousarray(np.asarray(Q, dtype=np.float32))
    K = np.ascontiguousarray(np.asarray(K, dtype=np.float32))
    V = np.ascontiguousarray(np.asarray(V, dtype=np.float32))
    omega = np.asarray(omega, dtype=np.float32)

    QT = Q.T.astype(bf)
    KT = K.T.astype(bf)
    omega_s = (omega * np.float32(D ** -0.25)).astype(bf)
    hscale = np.float32(1.0 / (2.0 * math.sqrt(D)))
    hq = (Q * Q).sum(axis=1) * hscale            # [N]
    hk = (K * K).sum(axis=1) * hscale
    ehk = np.exp(-hk).astype(np.float32)          # [N]
    Vb = V.astype(bf).astype(np.float32)
    VP = (ehk[:, None] * Vb).astype(bf)           # V' rows scaled
    # eps * per-segment colsum of raw [V] (bf16-rounded V)
    nseg_tot = N_CORES * NSEG
    cvs = (EPS_PHI * Vb.reshape(nseg_tot, 512, DV).sum(axis=1)).astype(bf)
    ident = np.eye(P, dtype=np.float32).astype(bf)

    def cols(x):   # [N] -> per-core [P, NSEG*CH] with x[s*512+c*128+p]
        return np.ascontiguousarray(
            x.reshape(N_CORES, NSEG, CH, P).transpose(0, 3, 1, 2)
            .reshape(N_CORES, P, NSEG * CH))

    hqm = cols(-hq).astype(np.float32)
    ehkc = cols(ehk).astype(bf)

    in_maps = []
    for c in range(N_CORES):
        sl = slice(c * TOK, (c + 1) * TOK)
        in_maps.append({
            "QT": np.ascontiguousarray(QT[:, sl]),
            "KT": np.ascontiguousarray(KT[:, sl]),
            "VP": VP[sl],
            "omega": omega_s,
            "HQM": hqm[c],
            "EHK": ehkc[c],
            "CVS": np.ascontiguousarray(
                cvs[c * NSEG:(c + 1) * NSEG].reshape(1, NSEG * DV)),
            "identb": ident,
        })
    return in_maps


def assemble(results):
    outs = []
    for c in range(N_CORES):
        r = results[c]
        num = np.asarray(r["num"], dtype=np.float32)          # [TOK, 256]
        den = r["den"].reshape(P, NSEG, CH).transpose(1, 2, 0).reshape(TOK)
        smax = r["smax"].reshape(NSEG)                        # e^{segmax}
        den = den + (M * EPS_NORM) * np.repeat(smax, 512)
        outs.append(num / den[:, None])
    return np.concatenate(outs, axis=0).astype(np.float32)


def kernel(Q, K, V, omega, num_batch, batch_seg):
    nc = _get_nc()
    in_maps = make_in_maps(Q, K, V, omega)
    res = run_bass_kernel_spmd(nc, in_maps, core_ids=list(range(N_CORES)))
    return assemble(res.results)
